# revision 1
# baseline (speedup 1.0000x reference)
import math

import numpy as np

N_ATOMS = 1_000_000
D = 128
HID = 128
NE = 4
NCORES = 8
CH = N_ATOMS // NCORES
BLK = 512
M2 = 16
CHUNK = 16

_prog_cache = {}


def _build_program(TOT, expert_of_block):
    import concourse.bacc as bacc
    import concourse.mybir as mybir
    import concourse.tile as tile

    f32 = mybir.dt.float32
    GY = math.ceil(TOT / M2)

    nc = bacc.Bacc("TRN2", target_bir_lowering=False, debug=False)
    x = nc.dram_tensor("x", [128, TOT * BLK], f32, kind="ExternalInput")
    w1 = nc.dram_tensor("w1", [128, NE * HID], f32, kind="ExternalInput")
    b1t = nc.dram_tensor("b1t", [128, NE], f32, kind="ExternalInput")
    w2s = nc.dram_tensor("w2s", [128, NE * M2 * M2], f32, kind="ExternalInput")
    y = nc.dram_tensor("y", [M2, GY * BLK], f32, kind="ExternalOutput")

    relu = mybir.ActivationFunctionType.Relu

    with tile.TileContext(nc) as tc:
        with (
            tc.tile_pool(name="const", bufs=1) as cpool,
            tc.tile_pool(name="xin", bufs=3) as xpool,
            tc.tile_pool(name="hsb", bufs=4) as spool,
            tc.tile_pool(name="hps", bufs=3, space="PSUM") as hpool,
            tc.tile_pool(name="yps", bufs=2, space="PSUM") as ypool,
            tc.tile_pool(name="osb", bufs=3) as opool,
        ):
            w1sb = cpool.tile([128, NE * HID], f32)
            nc.sync.dma_start(w1sb[:], w1[:])
            b1sb = cpool.tile([128, NE], f32)
            nc.sync.dma_start(b1sb[:], b1t[:])
            w2sb = cpool.tile([128, NE * M2 * M2], f32)
            nc.sync.dma_start(w2sb[:], w2s[:])

            xt = None
            py = None
            osb = None
            for b in range(TOT):
                e = int(expert_of_block[b])
                j = b % M2
                g = b // M2
                kb = b % CHUNK
                if kb == 0:
                    nblk = min(CHUNK, TOT - b)
                    xt = xpool.tile([128, CHUNK * BLK], f32)
                    nc.sync.dma_start(
                        xt[:, : nblk * BLK],
                        x[:, b * BLK : (b + nblk) * BLK],
                    )
                ph = hpool.tile([128, BLK], f32)
                nc.tensor.matmul(
                    ph[:],
                    w1sb[:, e * HID : (e + 1) * HID],
                    xt[:, kb * BLK : (kb + 1) * BLK],
                    start=True,
                    stop=True,
                )
                hs = spool.tile([128, BLK], f32)
                nc.scalar.activation(hs[:], ph[:], relu, bias=b1sb[:, e : e + 1])
                if j == 0:
                    py = ypool.tile([M2, BLK], f32)
                last = b == TOT - 1
                nc.tensor.matmul(
                    py[:],
                    w2sb[:, (e * M2 + j) * M2 : (e * M2 + j + 1) * M2],
                    hs[:],
                    start=(j == 0),
                    stop=(j == M2 - 1 or last),
                )
                if j == M2 - 1 or last:
                    osb = opool.tile([M2, BLK], f32)
                    nc.vector.tensor_copy(osb[:], py[:])
                    nc.gpsimd.dma_start(y[:, g * BLK : (g + 1) * BLK], osb[:])

    nc.finalize()
    return nc


def kernel(element, desc, W1, b1, W2, b2):
    from concourse.bass_utils import run_bass_kernel_spmd

    el = np.asarray(element)
    ds = np.asarray(desc, dtype=np.float32)
    W1 = np.asarray(W1, dtype=np.float32)
    b1 = np.asarray(b1, dtype=np.float32)
    W2 = np.asarray(W2, dtype=np.float32)
    b2 = np.asarray(b2, dtype=np.float32)

    orders = []
    counts = np.zeros((NCORES, NE), dtype=np.int64)
    for c in range(NCORES):
        ec = el[c * CH : (c + 1) * CH]
        orders.append(np.argsort(ec, kind="stable"))
        counts[c] = np.bincount(ec, minlength=NE)

    caps = np.ceil(counts.max(axis=0) / BLK).astype(np.int64)
    offs = np.concatenate([[0], np.cumsum(caps)])
    TOT = int(offs[-1])
    GY = math.ceil(TOT / M2)
    expert_of_block = np.repeat(np.arange(NE), caps)

    key = (TOT, expert_of_block.tobytes())
    nc = _prog_cache.get(key)
    if nc is None:
        nc = _build_program(TOT, expert_of_block)
        _prog_cache[key] = nc

    w1cat = np.ascontiguousarray(W1.transpose(1, 0, 2).reshape(128, NE * HID))
    b1T = np.ascontiguousarray(b1.T)
    w2sel = np.zeros((128, NE, M2, M2), dtype=np.float32)
    for j in range(M2):
        w2sel[:, :, j, j] = W2.T
    w2sel = w2sel.reshape(128, NE * M2 * M2)

    in_maps = []
    for c in range(NCORES):
        xc = np.zeros((128, TOT * BLK), dtype=np.float32)
        dsort = ds[c * CH : (c + 1) * CH][orders[c]]
        pos = 0
        for e in range(NE):
            cnt = int(counts[c, e])
            xc[:, offs[e] * BLK : offs[e] * BLK + cnt] = dsort[pos : pos + cnt].T
            pos += cnt
        in_maps.append({"x": xc, "w1": w1cat, "b1t": b1T, "w2s": w2sel})

    res = run_bass_kernel_spmd(nc, in_maps, list(range(NCORES)))

    out = np.empty(N_ATOMS, dtype=np.float32)
    for c in range(NCORES):
        y_np = np.asarray(res.results[c]["y"])
        blocks = (
            y_np.reshape(M2, GY, BLK).transpose(1, 0, 2).reshape(GY * M2, BLK)
        )
        res_sorted = np.empty(CH, dtype=np.float32)
        pos = 0
        for e in range(NE):
            cnt = int(counts[c, e])
            seg = blocks[offs[e] : offs[e] + caps[e]].reshape(-1)[:cnt]
            res_sorted[pos : pos + cnt] = seg + b2[e]
            pos += cnt
        tmp = np.empty(CH, dtype=np.float32)
        tmp[orders[c]] = res_sorted
        out[c * CH : (c + 1) * CH] = tmp
    return out


# revision 9
# speedup vs baseline: 2.6754x; 2.6754x over previous
import math

import numpy as np

N_ATOMS = 1_000_000
D = 128
HID = 128
NE = 4
NCORES = 8
CH = N_ATOMS // NCORES
BLK = 512
M2 = 16
CHUNK = 16

_prog_cache = {}


def _build_program(TOT, expert_of_block):
    import concourse.bacc as bacc
    import concourse.mybir as mybir
    import concourse.tile as tile

    f32 = mybir.dt.float32
    f32r = mybir.dt.float32r
    GY = math.ceil(TOT / M2)

    nc = bacc.Bacc("TRN2", target_bir_lowering=False, debug=False)
    x = nc.dram_tensor("x", [128, TOT * BLK], f32r, kind="ExternalInput")
    w1 = nc.dram_tensor("w1", [128, NE * HID], f32r, kind="ExternalInput")
    b1t = nc.dram_tensor("b1t", [128, NE], f32, kind="ExternalInput")
    w2s = nc.dram_tensor("w2s", [128, NE * M2 * M2], f32r, kind="ExternalInput")
    y = nc.dram_tensor("y", [M2, GY * BLK], f32, kind="ExternalOutput")

    relu = mybir.ActivationFunctionType.Relu

    with tile.TileContext(nc) as tc:
        with (
            tc.tile_pool(name="const", bufs=1) as cpool,
            tc.tile_pool(name="xin", bufs=3) as xpool,
            tc.tile_pool(name="hsb", bufs=4) as spool,
            tc.tile_pool(name="hps", bufs=3, space="PSUM") as hpool,
            tc.tile_pool(name="yps", bufs=2, space="PSUM") as ypool,
            tc.tile_pool(name="osb", bufs=3) as opool,
        ):
            w1sb = cpool.tile([128, NE * HID], f32r)
            nc.sync.dma_start(w1sb[:], w1[:])
            b1sb = cpool.tile([128, NE], f32)
            nc.sync.dma_start(b1sb[:], b1t[:])
            w2sb = cpool.tile([128, NE * M2 * M2], f32r)
            nc.sync.dma_start(w2sb[:], w2s[:])

            xt = None
            py = None
            osb = None
            for b in range(TOT):
                e = int(expert_of_block[b])
                j = b % M2
                g = b // M2
                kb = b % CHUNK
                if kb == 0:
                    nblk = min(CHUNK, TOT - b)
                    xt = xpool.tile([128, CHUNK * BLK], f32r)
                    nc.sync.dma_start(
                        xt[:, : nblk * BLK],
                        x[:, b * BLK : (b + nblk) * BLK],
                    )
                ph = hpool.tile([128, BLK], f32)
                nc.tensor.matmul(
                    ph[:],
                    w1sb[:, e * HID : (e + 1) * HID],
                    xt[:, kb * BLK : (kb + 1) * BLK],
                    start=True,
                    stop=True,
                )
                hs = spool.tile([128, BLK], f32r)
                nc.scalar.activation(hs[:], ph[:], relu, bias=b1sb[:, e : e + 1])
                if j == 0:
                    py = ypool.tile([M2, BLK], f32)
                last = b == TOT - 1
                nc.tensor.matmul(
                    py[:],
                    w2sb[:, (e * M2 + j) * M2 : (e * M2 + j + 1) * M2],
                    hs[:],
                    start=(j == 0),
                    stop=(j == M2 - 1 or last),
                )
                if j == M2 - 1 or last:
                    osb = opool.tile([M2, BLK], f32)
                    nc.vector.tensor_copy(osb[:], py[:])
                    nc.gpsimd.dma_start(y[:, g * BLK : (g + 1) * BLK], osb[:])

    nc.finalize()
    return nc


def kernel(element, desc, W1, b1, W2, b2):
    from concourse.bass_utils import run_bass_kernel_spmd

    el = np.asarray(element)
    ds = np.asarray(desc, dtype=np.float32)
    W1 = np.asarray(W1, dtype=np.float32)
    b1 = np.asarray(b1, dtype=np.float32)
    W2 = np.asarray(W2, dtype=np.float32)
    b2 = np.asarray(b2, dtype=np.float32)

    orders = []
    counts = np.zeros((NCORES, NE), dtype=np.int64)
    for c in range(NCORES):
        ec = el[c * CH : (c + 1) * CH]
        orders.append(np.argsort(ec, kind="stable"))
        counts[c] = np.bincount(ec, minlength=NE)

    caps = np.ceil(counts.max(axis=0) / BLK).astype(np.int64)
    offs = np.concatenate([[0], np.cumsum(caps)])
    TOT = int(offs[-1])
    GY = math.ceil(TOT / M2)
    expert_of_block = np.repeat(np.arange(NE), caps)

    key = (TOT, expert_of_block.tobytes())
    nc = _prog_cache.get(key)
    if nc is None:
        nc = _build_program(TOT, expert_of_block)
        _prog_cache[key] = nc

    w1cat = np.ascontiguousarray(W1.transpose(1, 0, 2).reshape(128, NE * HID))
    b1T = np.ascontiguousarray(b1.T)
    w2sel = np.zeros((128, NE, M2, M2), dtype=np.float32)
    for j in range(M2):
        w2sel[:, :, j, j] = W2.T
    w2sel = w2sel.reshape(128, NE * M2 * M2)

    in_maps = []
    for c in range(NCORES):
        xc = np.zeros((128, TOT * BLK), dtype=np.float32)
        dsort = ds[c * CH : (c + 1) * CH][orders[c]]
        pos = 0
        for e in range(NE):
            cnt = int(counts[c, e])
            xc[:, offs[e] * BLK : offs[e] * BLK + cnt] = dsort[pos : pos + cnt].T
            pos += cnt
        in_maps.append({"x": xc, "w1": w1cat, "b1t": b1T, "w2s": w2sel})

    res = run_bass_kernel_spmd(nc, in_maps, list(range(NCORES)))

    out = np.empty(N_ATOMS, dtype=np.float32)
    for c in range(NCORES):
        y_np = np.asarray(res.results[c]["y"])
        blocks = (
            y_np.reshape(M2, GY, BLK).transpose(1, 0, 2).reshape(GY * M2, BLK)
        )
        res_sorted = np.empty(CH, dtype=np.float32)
        pos = 0
        for e in range(NE):
            cnt = int(counts[c, e])
            seg = blocks[offs[e] : offs[e] + caps[e]].reshape(-1)[:cnt]
            res_sorted[pos : pos + cnt] = seg + b2[e]
            pos += cnt
        tmp = np.empty(CH, dtype=np.float32)
        tmp[orders[c]] = res_sorted
        out[c * CH : (c + 1) * CH] = tmp
    return out
